# revision 1
# baseline (speedup 1.0000x reference)
"""Trainium2 Bass kernel for IntMultiPrecConv2d (moe_routing).

Math reduction: the two routing masks (argmax one-hot over 2 classes) are
complementary, so the module is exactly

    out[b, c] = scale[c] * conv2d(x, weight)[b, c] + bias[c]

with per-channel scale/bias computed on the host from the routing and the
int-quant parameters.

Device: 3x3 pad-1 conv as shifted matmuls accumulating in PSUM (Cin=128 on
the PE contraction dim, Cout=256 as two 128-wide tiles), then per-channel
scale+bias on eviction (ScalarE Identity activation with per-partition
scale/bias operands).

Speed: inputs/weights in fp8-e4m3; 8 of the 9 conv taps run as 4
DoubleRow matmuls (two taps packed per PE cell -> 0.5 cycles/row), the 9th
as a plain fp8 matmul. DoubleRow needs the rhs pair stride to be a
multiple of 16 bytes, so the padded image is replicated inside one SBUF
tile at offsets D1/D2 chosen to make each tap pair's stride %16==0.
Accumulation is fp32 in PSUM; the output (bias-dominated, which dilutes
the fp8 conv error to ~1e-5 relative) is written back in fp32.

Sharding: data-parallel over batch, 8 cores x 4 images.
"""

import numpy as np
import ml_dtypes

B, CIN, COUT, H, W = 32, 128, 256, 56, 56
NCORES = 8
BPC = B // NCORES          # images per core
WP = W + 2                 # padded width 58
HP = H + 2                 # padded height 58
XLEN = HP * WP + 4         # padded image elems per channel (+ slack)
ROWS = 8                   # output rows per PSUM chunk
NCHUNK = H // ROWS         # 7
CH = ROWS * W              # 448 valid output pixels per chunk
OUTN = H * W               # 3136
D1, D2 = 3375, 6744        # replica offsets: pair strides D1+1, D2+56 %16==0
XTOT = D2 + XLEN
# DoubleRow tap pairs (first_tap, second_tap, replica_base): stride =
# base + off(second) - off(first) where off(k) = (k//3)*WP + k%3.
PAIRS = [(0, 1, D1), (4, 5, D1), (6, 7, D1), (2, 3, D2)]

_CACHE = {}


def _build_bass():
    import concourse.bass as bass
    import concourse.tile as tile
    import concourse.mybir as mybir
    from concourse import bacc

    f8 = mybir.dt.float8e4
    f32 = mybir.dt.float32
    bf16 = mybir.dt.bfloat16
    AF = mybir.ActivationFunctionType

    def mk_ap(proto, steps_counts):
        # Hand-built access pattern (same tensor/offset/partition-pitch as
        # proto): needed for the DoubleRow pair dim, whose stride spans
        # replica copies and can't be expressed through rearrange/slicing.
        return bass.AP(proto.tensor, proto.offset,
                       [list(proto.ap[0])] + [list(p) for p in steps_counts])

    nc = bacc.Bacc("TRN2", target_bir_lowering=False, debug=False,
                   num_devices=NCORES)
    xp = nc.dram_tensor("xp", (BPC, CIN, XLEN), f8, kind="ExternalInput").ap()
    wt = nc.dram_tensor("wt", (CIN, 4 * 512 + 256), f8,
                        kind="ExternalInput").ap()
    sc = nc.dram_tensor("scale", (2, CIN, 1), f32, kind="ExternalInput").ap()
    bi = nc.dram_tensor("bias", (2, CIN, 1), f32, kind="ExternalInput").ap()
    out = nc.dram_tensor("out", (BPC, COUT, OUTN), f32,
                         kind="ExternalOutput").ap()

    with tile.TileContext(nc) as tc:
        with (
            tc.tile_pool(name="wpool", bufs=1) as wpool,
            tc.tile_pool(name="bpool", bufs=1) as bpool,
            tc.tile_pool(name="spool", bufs=1) as spool,
            tc.tile_pool(name="xpool", bufs=2) as xpool,
            tc.tile_pool(name="opool", bufs=6) as opool,
            tc.tile_pool(name="pspool", bufs=8, space="PSUM") as pspool,
        ):
            wtile = wpool.tile([128, 4 * 512 + 256], f8)
            nc.sync.dma_start(wtile[:], wt[:, :])
            btile = bpool.tile([128, 4], f32)
            for half in range(2):
                nc.sync.dma_start(btile[:, half:half + 1], bi[half])
                nc.sync.dma_start(btile[:, 2 + half:3 + half], sc[half])

            # PE warmup while the first x DMA is in flight (the cost of the
            # clock ramp is paid on dummy matmuls instead of real ones).
            scr = spool.tile([128, CH], bf16)
            nc.vector.memset(scr[:], 0.0)
            wps = pspool.tile([128, CH], f32, tag="ps")
            for _ in range(4):
                nc.tensor.matmul(wps[:], scr[:, :128], scr[:],
                                 start=True, stop=True)

            for b in range(BPC):
                xt = xpool.tile([128, XTOT], f8)
                nc.sync.dma_start(xt[:, :XLEN], xp[b])
                # replica copies for the DoubleRow pair strides (SBUF->SBUF)
                nc.sync.dma_start(xt[:, D1:D1 + HP * WP], xt[:, 0:HP * WP])
                nc.sync.dma_start(xt[:, D2:D2 + HP * WP], xt[:, 0:HP * WP])
                for half in range(2):
                    for j in range(NCHUNK):
                        grow = ROWS * j
                        ps = pspool.tile([128, CH], f32, tag="ps")
                        for mi, (k1, k2, base) in enumerate(PAIRS):
                            kh, kw = divmod(k1, 3)
                            off = (grow + kh) * WP + kw
                            d = base + (k2 // 3) * WP + k2 % 3 - kh * WP - kw
                            rhs = mk_ap(xt[:, off:off + 1],
                                        [[d, 2], [WP, ROWS], [1, W]])
                            lhsT = mk_ap(
                                wtile[:, 512 * mi + 128 * half:
                                      512 * mi + 128 * half + 1],
                                [[256, 2], [1, 128]])
                            nc.tensor.matmul(
                                ps[:], lhsT, rhs, start=(mi == 0), stop=False,
                                perf_mode=mybir.MatmulPerfMode.DoubleRow)
                        # single tap k8 = (2, 2), plain fp8 matmul
                        off = (grow + 2) * WP + 2
                        rhs = mk_ap(xt[:, off:off + 1],
                                    [[WP, ROWS], [1, W]])
                        nc.tensor.matmul(
                            ps[:], wtile[:, 2048 + 128 * half:
                                         2048 + 128 * half + 128],
                            rhs, start=False, stop=True)
                        oc = opool.tile([128, CH], f32)
                        nc.scalar.activation(oc[:], ps[:], AF.Identity,
                                             bias=btile[:, half:half + 1],
                                             scale=btile[:, 2 + half:3 + half])
                        nc.sync.dma_start(
                            out[b, half * 128:half * 128 + 128,
                                j * CH:(j + 1) * CH],
                            oc[:])
    nc.compile()
    return nc


def _prep(x, weight, alpha_weight, alpha2, b8_2, nb_2, nsh_2, alpha8, b16_8,
          nsh_8):
    """Host-side: routing -> per-channel scale/bias; pack fp8 weights in
    DoubleRow pair layout; zero-pad + fp8-cast x."""
    f64 = np.float64
    sel = np.argmax(np.asarray(alpha_weight), axis=0)
    sw0 = sel == 0
    scale = np.where(sw0,
                     np.asarray(alpha2, f64) * np.exp2(-np.asarray(nsh_2, f64)),
                     np.asarray(alpha8, f64) * np.exp2(-np.asarray(nsh_8, f64)))
    bias = np.where(
        sw0,
        np.asarray(b8_2, f64) * np.exp2(np.asarray(nb_2, f64) -
                                        np.asarray(nsh_2, f64)),
        np.asarray(alpha8, f64) * np.asarray(b16_8, f64) *
        np.exp2(-np.asarray(nsh_8, f64)))

    # wT[ci, k, co] = weight[co, ci, kh, kw], unscaled (fp8 dynamic range)
    wT = np.ascontiguousarray(
        np.asarray(weight, np.float32).transpose(1, 2, 3, 0).reshape(
            CIN, 9, COUT))
    wpk = np.zeros((CIN, 4 * 512 + 256), np.float32)
    for p, (k1, k2, _) in enumerate(PAIRS):
        wpk[:, 512 * p:512 * p + 256] = wT[:, k1]
        wpk[:, 512 * p + 256:512 * p + 512] = wT[:, k2]
    wpk[:, 2048:2304] = wT[:, 8]
    wpk = wpk.astype(ml_dtypes.float8_e4m3)

    xpad = np.zeros((B, CIN, XLEN), dtype=ml_dtypes.float8_e4m3)
    xv = xpad[:, :, :HP * WP].reshape(B, CIN, HP, WP)
    xv[:, :, 1:H + 1, 1:W + 1] = np.asarray(x)

    sc2 = np.ascontiguousarray(scale.astype(np.float32).reshape(2, 128, 1))
    bias2 = np.ascontiguousarray(bias.astype(np.float32).reshape(2, 128, 1))
    return xpad, wpk, sc2, bias2


def _run(inputs, trace=False, **spmd_kwargs):
    from concourse import bass_utils

    if "nc" not in _CACHE:
        _CACHE["nc"] = _build_bass()
    nc = _CACHE["nc"]

    xpad, wpk, sc2, bias2 = _prep(**inputs)
    in_maps = [
        {"xp": xpad[c * BPC:(c + 1) * BPC], "wt": wpk, "scale": sc2,
         "bias": bias2}
        for c in range(NCORES)
    ]
    res = bass_utils.run_bass_kernel_spmd(
        nc, in_maps, core_ids=list(range(NCORES)), trace=trace, **spmd_kwargs)
    parts = [r["out"].reshape(BPC, COUT, H, W) for r in res.results]
    return np.concatenate(parts, axis=0), res


def kernel(**inputs) -> np.ndarray:
    out, _ = _run(inputs, trace=False)
    return out



# revision 5
# speedup vs baseline: 1.4940x; 1.4940x over previous
"""Trainium2 Bass kernel for IntMultiPrecConv2d (moe_routing).

Math reduction: the two routing masks (argmax one-hot over 2 classes) are
complementary, so the module is exactly

    out[b, c] = scale[c] * conv2d(x, weight)[b, c] + bias[c]

with per-channel scale/bias computed on the host from the routing and the
int-quant parameters.

Device: 3x3 pad-1 conv as shifted matmuls accumulating in PSUM (Cin=128 on
the PE contraction dim, Cout=256 as two 128-wide tiles), then per-channel
scale+bias on eviction, split between the Activation and Vector engines.

Speed: inputs/weights in fp8-e4m3; ALL 9 conv taps run as 5 DoubleRow
matmuls (two taps packed per PE cell -> 0.5 cycles/output-row). The padded
row pitch is WP=66 and the image is replicated once inside the SBUF tile at
byte offset D=3838 (D % 16 == 14), which makes the tap pairs
(0,2),(3,5),(6,8) [stride D+2] and (1,4) [stride D+66] all 16-byte aligned
as DoubleRow requires. The odd 9th tap (7) is paired with all-zero weights
at in-image stride 16 -- the second row's data is multiplied by zero, so
any 16-aligned garbage works. Accumulation is fp32 in PSUM; outputs are
written back in bf16 (bias-dominated output -> ~1e-3 relative) and cast to
fp32 on the host.

Overlap: image 0 ships host-replicated via four row-quarter DMAs so the
first matmul starts ~3us in; images 1-3 ship base-only and the replica is
copied on-device by the (otherwise idle) GpSimd engine, keeping the serial
DMA bus for the real input/output traffic. ~10 warmup matmuls hold the PE
clock ramp until real work arrives. Output DMA is batched per
(image, cout-half), except the last half which streams per-chunk to
shorten the tail.

Sharding: data-parallel over batch, 8 cores x 4 images.
"""

import numpy as np
import ml_dtypes

B, CIN, COUT, H, W = 32, 128, 256, 56, 56
NCORES = 8
BPC = B // NCORES          # images per core
WP = 66                    # padded row pitch (W+2 data cols + 8 slack)
HP = H + 2                 # padded height 58
IMG = HP * WP              # 3828 bytes (fp8) per channel per copy
D = 3838                   # replica byte offset; D % 16 == 14
XTOT = D + IMG             # 7666
XPAD = 7680                # tile width, 16-aligned
ROWS = 8                   # output rows per PSUM chunk
NCHUNK = H // ROWS         # 7
CH = ROWS * W              # 448 output pixels per chunk
OUTN = H * W               # 3136
# DoubleRow pairs (k1, k2, pair_byte_stride); k2 None -> zero-weight pair.
# off(k) = (k//3)*WP + k%3; stride = D + off(k2) - off(k1) for replica
# pairs, 16 for the zero pair.
PAIRS = [(0, 2, D + 2), (3, 5, D + 2), (6, 8, D + 2), (1, 4, D + WP),
         (7, None, 16)]
NWARM = 10

_CACHE = {}


def _build_bass():
    import concourse.bass as bass
    import concourse.tile as tile
    import concourse.mybir as mybir
    from concourse import bacc

    f8 = mybir.dt.float8e4
    f32 = mybir.dt.float32
    bf16 = mybir.dt.bfloat16
    i16 = mybir.dt.int16
    AF = mybir.ActivationFunctionType
    ALU = mybir.AluOpType

    def mk_ap(proto, steps_counts):
        # Hand-built access pattern (same tensor/offset/partition-pitch as
        # proto): needed for the DoubleRow pair dim and the paired
        # base+replica DMAs, whose strides can't be expressed through
        # rearrange/slicing.
        return bass.AP(proto.tensor, proto.offset,
                       [list(proto.ap[0])] + [list(p) for p in steps_counts])

    nc = bacc.Bacc("TRN2", target_bir_lowering=False, debug=False,
                   num_devices=NCORES)
    xp = nc.dram_tensor("xp", (BPC, CIN, XPAD), f8, kind="ExternalInput").ap()
    wt = nc.dram_tensor("wt", (CIN, 5 * 512), f8, kind="ExternalInput").ap()
    sb = nc.dram_tensor("sb", (CIN, 4), f32, kind="ExternalInput").ap()
    out = nc.dram_tensor("out", (BPC, COUT, OUTN), bf16,
                         kind="ExternalOutput").ap()

    # b0 row-quarter boundaries (in bytes; the last quarter runs past the
    # image through the [IMG, D) gap, which the zero-weight pair's +16
    # shifted read touches at the bottom-right corner -- it must hold
    # real (host-zeroed) bytes, not SBUF garbage, since fp8 NaN*0 = NaN).
    QBYTES = [(0, 16 * WP), (16 * WP, 32 * WP), (32 * WP, 48 * WP),
              (48 * WP, D)]

    with tile.TileContext(nc) as tc:
        with (
            tc.tile_pool(name="wpool", bufs=1) as wpool,
            tc.tile_pool(name="bpool", bufs=1) as bpool,
            tc.tile_pool(name="spool", bufs=1) as spool,
            tc.tile_pool(name="xpool", bufs=4) as xpool,
            tc.tile_pool(name="opool", bufs=4) as opool,
            tc.tile_pool(name="pspool", bufs=8, space="PSUM") as pspool,
        ):
            # PE warmup scratch + matmuls: hold the clock ramp while the
            # first input DMAs are in flight.
            scr = spool.tile([128, CH], bf16)
            nc.vector.memset(scr[:], 0.0)
            wps = pspool.tile([128, CH], f32, tag="ps")
            for _ in range(NWARM):
                nc.tensor.matmul(wps[:], scr[:, :128], scr[:],
                                 start=True, stop=True)

            xts = [xpool.tile([128, XPAD], f8, name=f"xt{b}")
                   for b in range(BPC)]
            wtile = wpool.tile([128, 5 * 512], f8)
            btile = bpool.tile([128, 4], f32)

            # --- input DMAs, all on the SP queue ---
            # b0: four quarter DMAs, each moving the SAME rows of base and
            # host-built replica in one strided transfer, so chunk 0 can
            # start as soon as quarter 0 + weights land.
            q0, q1 = QBYTES[0]
            nc.sync.dma_start(
                mk_ap(xts[0][:, q0:q0 + 1], [[D, 2], [1, q1 - q0]]),
                mk_ap(xp[0][:, q0:q0 + 1], [[D, 2], [1, q1 - q0]]))
            nc.sync.dma_start(wtile[:], wt[:, :])
            for q0, q1 in QBYTES[1:]:
                nc.sync.dma_start(
                    mk_ap(xts[0][:, q0:q0 + 1], [[D, 2], [1, q1 - q0]]),
                    mk_ap(xp[0][:, q0:q0 + 1], [[D, 2], [1, q1 - q0]]))
            nc.sync.dma_start(btile[:], sb[:, :])
            # b1-3: base image only (through the gap, see above); replica
            # copied on-device by GpSimd.
            for b in range(1, BPC):
                nc.sync.dma_start(xts[b][:, :D], xp[b][:, :D])
            for b in range(1, BPC):
                nc.gpsimd.tensor_scalar(
                    xts[b][:, D:D + IMG].bitcast(i16),
                    xts[b][:, 0:IMG].bitcast(i16),
                    0, None, ALU.add)

            # --- main conv loop ---
            for b in range(BPC):
                xt = xts[b]
                for half in range(2):
                    last = (b == BPC - 1 and half == 1)
                    ot = opool.tile([128, OUTN], bf16)
                    for j in range(NCHUNK):
                        grow = ROWS * j
                        ps = pspool.tile([128, CH], f32, tag="ps")
                        for mi, (k1, k2, stride) in enumerate(PAIRS):
                            kh, kw = divmod(k1, 3)
                            off = (grow + kh) * WP + kw
                            rhs = mk_ap(xt[:, off:off + 1],
                                        [[stride, 2], [WP, ROWS], [1, W]])
                            lhsT = mk_ap(
                                wtile[:, 512 * mi + 128 * half:
                                      512 * mi + 128 * half + 1],
                                [[256, 2], [1, 128]])
                            nc.tensor.matmul(
                                ps[:], lhsT, rhs, start=(mi == 0),
                                stop=(mi == len(PAIRS) - 1),
                                perf_mode=mybir.MatmulPerfMode.DoubleRow)
                        osl = ot[:, j * CH:(j + 1) * CH]
                        if j % 2 == 0:
                            nc.scalar.activation(
                                osl, ps[:], AF.Identity,
                                bias=btile[:, half:half + 1],
                                scale=btile[:, 2 + half:3 + half])
                        else:
                            nc.vector.tensor_scalar(
                                osl, ps[:],
                                btile[:, 2 + half:3 + half],
                                btile[:, half:half + 1],
                                ALU.mult, ALU.add)
                        if last:
                            nc.sync.dma_start(
                                out[b, half * 128:half * 128 + 128,
                                    j * CH:(j + 1) * CH], osl)
                    if not last:
                        nc.sync.dma_start(
                            out[b, half * 128:half * 128 + 128, :], ot[:])
    nc.compile()
    return nc


def _prep(x, weight, alpha_weight, alpha2, b8_2, nb_2, nsh_2, alpha8, b16_8,
          nsh_8):
    """Host-side: routing -> per-channel scale/bias; pack fp8 weights in
    DoubleRow pair layout; zero-pad + fp8-cast x (replica for image 0 of
    each core's shard)."""
    f64 = np.float64
    sel = np.argmax(np.asarray(alpha_weight), axis=0)
    sw0 = sel == 0
    scale = np.where(sw0,
                     np.asarray(alpha2, f64) * np.exp2(-np.asarray(nsh_2, f64)),
                     np.asarray(alpha8, f64) * np.exp2(-np.asarray(nsh_8, f64)))
    bias = np.where(
        sw0,
        np.asarray(b8_2, f64) * np.exp2(np.asarray(nb_2, f64) -
                                        np.asarray(nsh_2, f64)),
        np.asarray(alpha8, f64) * np.asarray(b16_8, f64) *
        np.exp2(-np.asarray(nsh_8, f64)))

    # wT[ci, k, co] = weight[co, ci, kh, kw], unscaled (fp8 dynamic range)
    wT = np.ascontiguousarray(
        np.asarray(weight, np.float32).transpose(1, 2, 3, 0).reshape(
            CIN, 9, COUT))
    wpk = np.zeros((CIN, 5 * 512), np.float32)
    for p, (k1, k2, _) in enumerate(PAIRS):
        wpk[:, 512 * p:512 * p + 256] = wT[:, k1]
        if k2 is not None:
            wpk[:, 512 * p + 256:512 * p + 512] = wT[:, k2]
    wpk = wpk.astype(ml_dtypes.float8_e4m3)

    xpad = np.zeros((B, CIN, XPAD), dtype=ml_dtypes.float8_e4m3)
    xv = xpad[:, :, :IMG].reshape(B, CIN, HP, WP)
    xv[:, :, 1:H + 1, 1:W + 1] = np.asarray(x)
    # replica for the first image of each core's 4-image shard
    xpad[0::BPC, :, D:D + IMG] = xpad[0::BPC, :, 0:IMG]

    # btile cols: [bias_h0, bias_h1, scale_h0, scale_h1]
    sbt = np.empty((CIN, 4), np.float32)
    sbt[:, 0] = bias[:128]
    sbt[:, 1] = bias[128:]
    sbt[:, 2] = scale[:128]
    sbt[:, 3] = scale[128:]
    return xpad, wpk, sbt


def _run(inputs, trace=False, **spmd_kwargs):
    from concourse import bass_utils

    if "nc" not in _CACHE:
        _CACHE["nc"] = _build_bass()
    nc = _CACHE["nc"]

    xpad, wpk, sbt = _prep(**inputs)
    in_maps = [
        {"xp": xpad[c * BPC:(c + 1) * BPC], "wt": wpk, "sb": sbt}
        for c in range(NCORES)
    ]
    res = bass_utils.run_bass_kernel_spmd(
        nc, in_maps, core_ids=list(range(NCORES)), trace=trace, **spmd_kwargs)
    parts = [np.asarray(r["out"]).astype(np.float32).reshape(BPC, COUT, H, W)
             for r in res.results]
    return np.concatenate(parts, axis=0), res


def kernel(**inputs) -> np.ndarray:
    out, _ = _run(inputs, trace=False)
    return out


# revision 10
# speedup vs baseline: 1.9029x; 1.2737x over previous
"""Trainium2 Bass kernel for IntMultiPrecConv2d (moe_routing).

Math reduction: the two routing masks (argmax one-hot over 2 classes) are
complementary, so the module is exactly

    out[b, c] = scale[c] * conv2d(x, weight)[b, c] + bias[c]

with per-channel scale/bias computed on the host from the routing and the
int-quant parameters.

Device: 3x3 pad-1 conv as shifted matmuls accumulating in PSUM (Cin=128 on
the PE contraction dim, Cout=256 as two 128-wide tiles), then per-channel
scale+bias on eviction, split between the Activation and Vector engines.

Speed: inputs/weights in fp8-e4m3; ALL 9 conv taps run as 5 DoubleRow
matmuls (two taps packed per PE cell -> 0.5 cycles/output-row). The padded
row pitch is WP=66 and the image is replicated once inside the SBUF tile at
byte offset D=3838 (D % 16 == 14), which makes the tap pairs
(0,2),(3,5),(6,8) [stride D+2] and (1,4) [stride D+66] all 16-byte aligned
as DoubleRow requires. The odd 9th tap (7) is paired with all-zero weights
at in-image stride 16 -- the second row's data is multiplied by zero, so
any 16-aligned garbage works. Accumulation is fp32 in PSUM; outputs are
written back in bf16 (bias-dominated output -> ~1e-3 relative) and cast to
fp32 on the host.

Overlap: image 0 ships host-replicated via four row-quarter DMAs so the
first matmul starts ~3us in; images 1-3 ship base-only and the replica is
copied on-device by the (otherwise idle) GpSimd engine, keeping the serial
DMA bus for the real input/output traffic. ~10 warmup matmuls hold the PE
clock ramp until real work arrives. Output DMA is batched per
(image, cout-half), except the last half which streams per-chunk to
shorten the tail.

Sharding: data-parallel over batch, 8 cores x 4 images.
"""

import numpy as np
import ml_dtypes

B, CIN, COUT, H, W = 32, 128, 256, 56, 56
NCORES = 8
BPC = B // NCORES          # images per core
WP = 66                    # padded row pitch (W+2 data cols + 8 slack)
HP = H + 2                 # padded height 58
IMG = HP * WP              # 3828 bytes (fp8) per channel per copy
D = 3838                   # replica byte offset; D % 16 == 14
XTOT = D + IMG             # 7666
XPAD = 7680                # tile width, 16-aligned
ROWS = 8                   # output rows per PSUM chunk
NCHUNK = H // ROWS         # 7
CH = ROWS * W              # 448 output pixels per chunk
OUTN = H * W               # 3136
# DoubleRow pairs (k1, k2, pair_byte_stride); k2 None -> zero-weight pair.
# off(k) = (k//3)*WP + k%3; stride = D + off(k2) - off(k1) for replica
# pairs, 16 for the zero pair.
PAIRS = [(0, 2, D + 2), (3, 5, D + 2), (6, 8, D + 2), (1, 4, D + WP),
         (7, None, 16)]
NWARM = 12

_CACHE = {}


def _build_bass():
    import concourse.bass as bass
    import concourse.tile as tile
    import concourse.mybir as mybir
    from concourse import bacc

    f8 = mybir.dt.float8e4
    f32 = mybir.dt.float32
    bf16 = mybir.dt.bfloat16
    i16 = mybir.dt.int16
    AF = mybir.ActivationFunctionType
    ALU = mybir.AluOpType

    def mk_ap(proto, steps_counts):
        # Hand-built access pattern (same tensor/offset/partition-pitch as
        # proto): needed for the DoubleRow pair dim and the paired
        # base+replica DMAs, whose strides can't be expressed through
        # rearrange/slicing.
        return bass.AP(proto.tensor, proto.offset,
                       [list(proto.ap[0])] + [list(p) for p in steps_counts])

    nc = bacc.Bacc("TRN2", target_bir_lowering=False, debug=False,
                   num_devices=NCORES)
    xp = nc.dram_tensor("xp", (BPC, CIN, XPAD), f8, kind="ExternalInput").ap()
    wt = nc.dram_tensor("wt", (CIN, 5 * 512), f8, kind="ExternalInput").ap()
    sb = nc.dram_tensor("sb", (CIN, 4), f32, kind="ExternalInput").ap()
    out = nc.dram_tensor("out", (BPC, COUT, OUTN), bf16,
                         kind="ExternalOutput").ap()

    # b0 load pieces: contiguous byte ranges ONLY. The dependency tracker
    # uses bounding spans, so a strided base+replica pair DMA would span
    # the whole tile and (a) WAW-chain the pieces serially and (b) make
    # every matmul wait for every piece. Base is split at padded row 32 so
    # chunks 0-2 only gate on the first half. Base pieces run through the
    # [IMG, D) gap, which the zero-weight pair's +16 shifted read touches
    # at the bottom-right corner -- it must hold real (host-zeroed) bytes,
    # not SBUF garbage, since fp8 NaN*0 = NaN in the PE.
    SPLIT = 32 * WP
    BPIECES = [(0, SPLIT), (D, D + SPLIT), (SPLIT, D), (D + SPLIT, XTOT)]

    with tile.TileContext(nc) as tc:
        with (
            tc.tile_pool(name="wpool", bufs=1) as wpool,
            tc.tile_pool(name="bpool", bufs=1) as bpool,
            tc.tile_pool(name="spool", bufs=1) as spool,
            tc.tile_pool(name="xpool", bufs=4) as xpool,
            tc.tile_pool(name="opool", bufs=4) as opool,
            tc.tile_pool(name="pspool", bufs=8, space="PSUM") as pspool,
        ):
            # PE warmup scratch + matmuls: hold the clock ramp while the
            # first input DMAs are in flight. Memset on GpSimd (idle at the
            # head; DVE memset would start the warmup ~0.7us later).
            scr = spool.tile([128, CH], bf16)
            nc.gpsimd.memset(scr[:], 0.0)
            wps = pspool.tile([128, CH], f32, tag="ps")
            for _ in range(NWARM):
                nc.tensor.matmul(wps[:], scr[:, :128], scr[:],
                                 start=True, stop=True)

            xts = [xpool.tile([128, XPAD], f8, name=f"xt{b}")
                   for b in range(BPC)]
            wtile = wpool.tile([128, 5 * 512], f8)
            btile = bpool.tile([128, 4], f32)

            # --- input DMAs, all on the SP queue ---
            # b0: four quarter DMAs, each moving the SAME rows of base and
            # host-built replica in one strided transfer, so chunk 0 can
            # start as soon as quarter 0 + weights land.
            for i, (q0, q1) in enumerate(BPIECES):
                nc.sync.dma_start(xts[0][:, q0:q1], xp[0][:, q0:q1])
                if i == 1:
                    nc.sync.dma_start(wtile[:], wt[:, :])
            nc.sync.dma_start(btile[:], sb[:, :])
            # b1-3: base image only (through the gap, see above); replica
            # copied on-device by GpSimd.
            for b in range(1, BPC):
                nc.sync.dma_start(xts[b][:, :D], xp[b][:, :D])
            for b in range(1, BPC):
                nc.gpsimd.tensor_scalar(
                    xts[b][:, D:D + IMG].bitcast(i16),
                    xts[b][:, 0:IMG].bitcast(i16),
                    0, None, ALU.add)

            # --- main conv loop ---
            for b in range(BPC):
                xt = xts[b]
                for half in range(2):
                    last = (b == BPC - 1 and half == 1)
                    ot = opool.tile([128, OUTN], bf16)
                    for j in range(NCHUNK):
                        grow = ROWS * j
                        ps = pspool.tile([128, CH], f32, tag="ps")
                        for mi, (k1, k2, stride) in enumerate(PAIRS):
                            kh, kw = divmod(k1, 3)
                            off = (grow + kh) * WP + kw
                            rhs = mk_ap(xt[:, off:off + 1],
                                        [[stride, 2], [WP, ROWS], [1, W]])
                            lhsT = mk_ap(
                                wtile[:, 512 * mi + 128 * half:
                                      512 * mi + 128 * half + 1],
                                [[256, 2], [1, 128]])
                            nc.tensor.matmul(
                                ps[:], lhsT, rhs, start=(mi == 0),
                                stop=(mi == len(PAIRS) - 1),
                                perf_mode=mybir.MatmulPerfMode.DoubleRow)
                        osl = ot[:, j * CH:(j + 1) * CH]
                        if j % 2 == 0:
                            nc.scalar.activation(
                                osl, ps[:], AF.Identity,
                                bias=btile[:, half:half + 1],
                                scale=btile[:, 2 + half:3 + half])
                        else:
                            nc.vector.tensor_scalar(
                                osl, ps[:],
                                btile[:, 2 + half:3 + half],
                                btile[:, half:half + 1],
                                ALU.mult, ALU.add)
                        if last and j in (1, 3, 5, 6):
                            # stream the tail out in pieces {0,1},{2,3},
                            # {4,5},{6} so the final transfer is one chunk
                            lo = (j - 1 if j % 2 else j) * CH
                            nc.sync.dma_start(
                                out[b, half * 128:half * 128 + 128,
                                    lo:(j + 1) * CH], ot[:, lo:(j + 1) * CH])
                    if not last:
                        nc.sync.dma_start(
                            out[b, half * 128:half * 128 + 128, :], ot[:])
    nc.compile()
    return nc


def _prep(x, weight, alpha_weight, alpha2, b8_2, nb_2, nsh_2, alpha8, b16_8,
          nsh_8):
    """Host-side: routing -> per-channel scale/bias; pack fp8 weights in
    DoubleRow pair layout; zero-pad + fp8-cast x (replica for image 0 of
    each core's shard)."""
    f64 = np.float64
    sel = np.argmax(np.asarray(alpha_weight), axis=0)
    sw0 = sel == 0
    scale = np.where(sw0,
                     np.asarray(alpha2, f64) * np.exp2(-np.asarray(nsh_2, f64)),
                     np.asarray(alpha8, f64) * np.exp2(-np.asarray(nsh_8, f64)))
    bias = np.where(
        sw0,
        np.asarray(b8_2, f64) * np.exp2(np.asarray(nb_2, f64) -
                                        np.asarray(nsh_2, f64)),
        np.asarray(alpha8, f64) * np.asarray(b16_8, f64) *
        np.exp2(-np.asarray(nsh_8, f64)))

    # wT[ci, k, co] = weight[co, ci, kh, kw], unscaled (fp8 dynamic range)
    wT = np.ascontiguousarray(
        np.asarray(weight, np.float32).transpose(1, 2, 3, 0).reshape(
            CIN, 9, COUT))
    wpk = np.zeros((CIN, 5 * 512), np.float32)
    for p, (k1, k2, _) in enumerate(PAIRS):
        wpk[:, 512 * p:512 * p + 256] = wT[:, k1]
        if k2 is not None:
            wpk[:, 512 * p + 256:512 * p + 512] = wT[:, k2]
    wpk = wpk.astype(ml_dtypes.float8_e4m3)

    xpad = np.zeros((B, CIN, XPAD), dtype=ml_dtypes.float8_e4m3)
    xv = xpad[:, :, :IMG].reshape(B, CIN, HP, WP)
    xv[:, :, 1:H + 1, 1:W + 1] = np.asarray(x)
    # replica for the first image of each core's 4-image shard
    xpad[0::BPC, :, D:D + IMG] = xpad[0::BPC, :, 0:IMG]

    # btile cols: [bias_h0, bias_h1, scale_h0, scale_h1]
    sbt = np.empty((CIN, 4), np.float32)
    sbt[:, 0] = bias[:128]
    sbt[:, 1] = bias[128:]
    sbt[:, 2] = scale[:128]
    sbt[:, 3] = scale[128:]
    return xpad, wpk, sbt


def _run(inputs, trace=False, **spmd_kwargs):
    from concourse import bass_utils

    if "nc" not in _CACHE:
        _CACHE["nc"] = _build_bass()
    nc = _CACHE["nc"]

    xpad, wpk, sbt = _prep(**inputs)
    in_maps = [
        {"xp": xpad[c * BPC:(c + 1) * BPC], "wt": wpk, "sb": sbt}
        for c in range(NCORES)
    ]
    res = bass_utils.run_bass_kernel_spmd(
        nc, in_maps, core_ids=list(range(NCORES)), trace=trace, **spmd_kwargs)
    parts = [np.asarray(r["out"]).astype(np.float32).reshape(BPC, COUT, H, W)
             for r in res.results]
    return np.concatenate(parts, axis=0), res


def kernel(**inputs) -> np.ndarray:
    out, _ = _run(inputs, trace=False)
    return out


# revision 15
# speedup vs baseline: 1.9452x; 1.0222x over previous
"""Trainium2 Bass kernel for IntMultiPrecConv2d (moe_routing).

Math reduction: the two routing masks (argmax one-hot over 2 classes) are
complementary, so the module is exactly

    out[b, c] = scale[c] * conv2d(x, weight)[b, c] + bias[c]

with per-channel scale/bias computed on the host from the routing and the
int-quant parameters.

Device: 3x3 pad-1 conv as shifted matmuls accumulating in PSUM (Cin=128 on
the PE contraction dim, Cout=256 as two 128-wide tiles), then per-channel
scale+bias on eviction, split between the Activation and Vector engines.

Speed: inputs/weights in fp8-e4m3; ALL 9 conv taps run as 5 DoubleRow
matmuls (two taps packed per PE cell -> 0.5 cycles/output-row). The padded
row pitch is WP=66 and the image is replicated once inside the SBUF tile at
byte offset D=3838 (D % 16 == 14), which makes the tap pairs
(0,2),(3,5),(6,8) [stride D+2] and (1,4) [stride D+66] all 16-byte aligned
as DoubleRow requires. The odd 9th tap (7) is paired with all-zero weights
at in-image stride 16 -- the second row's data is multiplied by zero, so
any 16-aligned garbage works. Accumulation is fp32 in PSUM; outputs are
written back in bf16 (bias-dominated output -> ~1e-3 relative) and cast to
fp32 on the host.

Overlap: image 0 ships host-replicated via four row-quarter DMAs so the
first matmul starts ~3us in; images 1-3 ship base-only and the replica is
copied on-device by the (otherwise idle) GpSimd engine, keeping the serial
DMA bus for the real input/output traffic. ~10 warmup matmuls hold the PE
clock ramp until real work arrives. Output DMA is batched per
(image, cout-half), except the last half which streams per-chunk to
shorten the tail.

Sharding: data-parallel over batch, 8 cores x 4 images.
"""

import numpy as np
import ml_dtypes

B, CIN, COUT, H, W = 32, 128, 256, 56, 56
NCORES = 8
BPC = B // NCORES          # images per core
WP = 66                    # padded row pitch (W+2 data cols + 8 slack)
HP = H + 2                 # padded height 58
IMG = HP * WP              # 3828 bytes (fp8) per channel per copy
D = 3838                   # replica byte offset; D % 16 == 14
XTOT = D + IMG             # 7666
XPAD = 7680                # tile width, 16-aligned
ROWS = 8                   # output rows per PSUM chunk
NCHUNK = H // ROWS         # 7
CH = ROWS * W              # 448 output pixels per chunk
OUTN = H * W               # 3136
# DoubleRow pairs (k1, k2, pair_byte_stride); k2 None -> zero-weight pair.
# off(k) = (k//3)*WP + k%3; stride = D + off(k2) - off(k1) for replica
# pairs, 16 for the zero pair.
PAIRS = [(0, 2, D + 2), (3, 5, D + 2), (6, 8, D + 2), (1, 4, D + WP),
         (7, None, 16)]
NWARM = 11

_CACHE = {}


def _build_bass():
    import concourse.bass as bass
    import concourse.tile as tile
    import concourse.mybir as mybir
    from concourse import bacc

    f8 = mybir.dt.float8e4
    f32 = mybir.dt.float32
    bf16 = mybir.dt.bfloat16
    i16 = mybir.dt.int16
    AF = mybir.ActivationFunctionType
    ALU = mybir.AluOpType

    def mk_ap(proto, steps_counts):
        # Hand-built access pattern (same tensor/offset/partition-pitch as
        # proto): needed for the DoubleRow pair dim and the paired
        # base+replica DMAs, whose strides can't be expressed through
        # rearrange/slicing.
        return bass.AP(proto.tensor, proto.offset,
                       [list(proto.ap[0])] + [list(p) for p in steps_counts])

    nc = bacc.Bacc("TRN2", target_bir_lowering=False, debug=False,
                   num_devices=NCORES)
    xp = nc.dram_tensor("xp", (BPC, CIN, XPAD), f8, kind="ExternalInput").ap()
    wt = nc.dram_tensor("wt", (CIN, 5 * 512), f8, kind="ExternalInput").ap()
    sb = nc.dram_tensor("sb", (CIN, 4), f32, kind="ExternalInput").ap()
    out = nc.dram_tensor("out", (BPC, COUT, OUTN), bf16,
                         kind="ExternalOutput").ap()

    # b0 load pieces: contiguous byte ranges ONLY. The dependency tracker
    # uses bounding spans, so a strided base+replica pair DMA would span
    # the whole tile and (a) WAW-chain the pieces serially and (b) make
    # every matmul wait for every piece. Every chunk's pair-AP span covers
    # the whole base region anyway (base tap -> replica tap), so base ships
    # as one piece; the replica is split at padded row 32 so chunks 0-2
    # gate only on its first half. Base runs through the [IMG, D) gap,
    # which the zero-weight pair's +16 shifted read touches at the
    # bottom-right corner -- it must hold real (host-zeroed) bytes, not
    # SBUF garbage, since fp8 NaN*0 = NaN in the PE.
    SPLIT = 32 * WP

    with tile.TileContext(nc) as tc:
        with (
            tc.tile_pool(name="wpool", bufs=1) as wpool,
            tc.tile_pool(name="bpool", bufs=1) as bpool,
            tc.tile_pool(name="spool", bufs=1) as spool,
            tc.tile_pool(name="xpool", bufs=4) as xpool,
            tc.tile_pool(name="opool", bufs=4) as opool,
            tc.tile_pool(name="pspool", bufs=8, space="PSUM") as pspool,
        ):
            # PE warmup scratch + matmuls: hold the clock ramp while the
            # first input DMAs are in flight. Memset on GpSimd (idle at the
            # head; DVE memset would start the warmup ~0.7us later).
            scr = spool.tile([128, CH], bf16)
            nc.gpsimd.memset(scr[:], 0.0)
            wps = pspool.tile([128, CH], f32, tag="ps")
            for _ in range(NWARM):
                nc.tensor.matmul(wps[:], scr[:, :128], scr[:],
                                 start=True, stop=True)

            xts = [xpool.tile([128, XPAD], f8, name=f"xt{b}")
                   for b in range(BPC)]
            wtile = wpool.tile([128, 5 * 512], f8)
            btile = bpool.tile([128, 4], f32)

            # --- input DMAs, all on the SP queue ---
            # b0: four quarter DMAs, each moving the SAME rows of base and
            # host-built replica in one strided transfer, so chunk 0 can
            # start as soon as quarter 0 + weights land.
            nc.sync.dma_start(xts[0][:, :D], xp[0][:, :D])
            nc.sync.dma_start(xts[0][:, D:D + SPLIT], xp[0][:, D:D + SPLIT])
            # pair-A weights first: chunk 0's first matmul gates on them
            nc.sync.dma_start(wtile[:, :512], wt[:, :512])
            nc.sync.dma_start(wtile[:, 512:], wt[:, 512:])
            nc.sync.dma_start(btile[:], sb[:, :])
            nc.sync.dma_start(xts[0][:, D + SPLIT:XTOT],
                              xp[0][:, D + SPLIT:XTOT])
            # b1-3: base image only (through the gap, see above); replica
            # copied on-device by GpSimd.
            for b in range(1, BPC):
                nc.sync.dma_start(xts[b][:, :D], xp[b][:, :D])
            for b in range(1, BPC):
                nc.gpsimd.tensor_scalar(
                    xts[b][:, D:D + IMG].bitcast(i16),
                    xts[b][:, 0:IMG].bitcast(i16),
                    0, None, ALU.add)

            # --- main conv loop ---
            for b in range(BPC):
                xt = xts[b]
                for half in range(2):
                    last = (b == BPC - 1)
                    ot = opool.tile([128, OUTN], bf16)
                    for j in range(NCHUNK):
                        grow = ROWS * j
                        ps = pspool.tile([128, CH], f32, tag="ps")
                        for mi, (k1, k2, stride) in enumerate(PAIRS):
                            kh, kw = divmod(k1, 3)
                            off = (grow + kh) * WP + kw
                            rhs = mk_ap(xt[:, off:off + 1],
                                        [[stride, 2], [WP, ROWS], [1, W]])
                            lhsT = mk_ap(
                                wtile[:, 512 * mi + 128 * half:
                                      512 * mi + 128 * half + 1],
                                [[256, 2], [1, 128]])
                            nc.tensor.matmul(
                                ps[:], lhsT, rhs, start=(mi == 0),
                                stop=(mi == len(PAIRS) - 1),
                                perf_mode=mybir.MatmulPerfMode.DoubleRow)
                        osl = ot[:, j * CH:(j + 1) * CH]
                        if j % 2 == 0:
                            nc.scalar.activation(
                                osl, ps[:], AF.Identity,
                                bias=btile[:, half:half + 1],
                                scale=btile[:, 2 + half:3 + half])
                        else:
                            nc.vector.tensor_scalar(
                                osl, ps[:],
                                btile[:, 2 + half:3 + half],
                                btile[:, half:half + 1],
                                ALU.mult, ALU.add)
                        if last and j in (2, 5, 6):
                            # stream b3's halves out in pieces {0-2},{3-5},
                            # {6} so the final bus transfer is one chunk
                            lo = {2: 0, 5: 3 * CH, 6: 6 * CH}[j]
                            nc.sync.dma_start(
                                out[b, half * 128:half * 128 + 128,
                                    lo:(j + 1) * CH], ot[:, lo:(j + 1) * CH])
                    if not last:
                        nc.sync.dma_start(
                            out[b, half * 128:half * 128 + 128, :], ot[:])
    nc.compile()
    return nc


def _prep(x, weight, alpha_weight, alpha2, b8_2, nb_2, nsh_2, alpha8, b16_8,
          nsh_8):
    """Host-side: routing -> per-channel scale/bias; pack fp8 weights in
    DoubleRow pair layout; zero-pad + fp8-cast x (replica for image 0 of
    each core's shard)."""
    f64 = np.float64
    sel = np.argmax(np.asarray(alpha_weight), axis=0)
    sw0 = sel == 0
    scale = np.where(sw0,
                     np.asarray(alpha2, f64) * np.exp2(-np.asarray(nsh_2, f64)),
                     np.asarray(alpha8, f64) * np.exp2(-np.asarray(nsh_8, f64)))
    bias = np.where(
        sw0,
        np.asarray(b8_2, f64) * np.exp2(np.asarray(nb_2, f64) -
                                        np.asarray(nsh_2, f64)),
        np.asarray(alpha8, f64) * np.asarray(b16_8, f64) *
        np.exp2(-np.asarray(nsh_8, f64)))

    # wT[ci, k, co] = weight[co, ci, kh, kw], unscaled (fp8 dynamic range)
    wT = np.ascontiguousarray(
        np.asarray(weight, np.float32).transpose(1, 2, 3, 0).reshape(
            CIN, 9, COUT))
    wpk = np.zeros((CIN, 5 * 512), np.float32)
    for p, (k1, k2, _) in enumerate(PAIRS):
        wpk[:, 512 * p:512 * p + 256] = wT[:, k1]
        if k2 is not None:
            wpk[:, 512 * p + 256:512 * p + 512] = wT[:, k2]
    wpk = wpk.astype(ml_dtypes.float8_e4m3)

    xpad = np.zeros((B, CIN, XPAD), dtype=ml_dtypes.float8_e4m3)
    xv = xpad[:, :, :IMG].reshape(B, CIN, HP, WP)
    xv[:, :, 1:H + 1, 1:W + 1] = np.asarray(x)
    # replica for the first image of each core's 4-image shard
    xpad[0::BPC, :, D:D + IMG] = xpad[0::BPC, :, 0:IMG]

    # btile cols: [bias_h0, bias_h1, scale_h0, scale_h1]
    sbt = np.empty((CIN, 4), np.float32)
    sbt[:, 0] = bias[:128]
    sbt[:, 1] = bias[128:]
    sbt[:, 2] = scale[:128]
    sbt[:, 3] = scale[128:]
    return xpad, wpk, sbt


def _run(inputs, trace=False, **spmd_kwargs):
    from concourse import bass_utils

    if "nc" not in _CACHE:
        _CACHE["nc"] = _build_bass()
    nc = _CACHE["nc"]

    xpad, wpk, sbt = _prep(**inputs)
    in_maps = [
        {"xp": xpad[c * BPC:(c + 1) * BPC], "wt": wpk, "sb": sbt}
        for c in range(NCORES)
    ]
    res = bass_utils.run_bass_kernel_spmd(
        nc, in_maps, core_ids=list(range(NCORES)), trace=trace, **spmd_kwargs)
    parts = [np.asarray(r["out"]).astype(np.float32).reshape(BPC, COUT, H, W)
             for r in res.results]
    return np.concatenate(parts, axis=0), res


def kernel(**inputs) -> np.ndarray:
    out, _ = _run(inputs, trace=False)
    return out
